# revision 12
# baseline (speedup 1.0000x reference)
"""Trainium2 Bass kernel for nn_BinaryMLP (BitNet-ternary SwiGLU MLP).

reference math (fp32):
    s_i = mean(|w_i|)            (per-tensor scalar, i in {1,3,2})
    wq_i = clip(round(w_i/s_i), -1, 1) * s_i     (ternary * scale)
    h1 = x @ w1q.T ; h3 = x @ w3q.T
    y  = (silu(h1) * h3) @ w2q.T

Strategy (8 cores, data-parallel over the 16384 tokens):
  - host: pad H 5461->5504, transpose x / w1 / w3 / w2 into contraction-major
    layouts (pure layout work, no arithmetic), split tokens 8 ways, and give
    each core a distinct 1/8 row-slice of each weight tensor.
  - device (per core, identical SPMD program):
      phase A (head, target <90us):
        * dummy AllReduce at t=0 absorbs the one-time CC rendezvous cost.
        * w1/w3 fp32 slices stream once into a resident SBUF tile on the SP
          DMA ring while DVE abs-reduces partial |w| sums per chunk; w2
          streams through a small staging pool (re-read later for its own
          ternarize, off the critical path).  x streams in parallel on the
          ACT DMA ring and is cast to bf16 on DVE.
        * tiny 8-float AllReduce -> ternarization thresholds +-s/2.
        * ternarize w1 (ACT Sign) / w3 (DVE is_ge/is_lt) from the resident
          fp32 copy -- no second DRAM read.  AllGather chunks are ordered
          smallest-first (2,3,6,8,12,12 h-tiles) so phase B starts on the
          first chunk ASAP; later chunks pipeline behind B's compute.
      phase B: h1/h3 matmuls vs resident bf16 x (fp8 ternary weights
        stationary, 8 psum banks), g = silu((s1/2) z1) * ((s3 s2/4) z3)
        -> bf16 -> DRAM.  w2's ternarize + AllGather + a partial prefetch
        of its fp8 tiles run in the shadow of B.
      phase C: y[m,d] = sum_h g[h,m] t2[h,d], g stationary, fp32 PSUM.
        gq loads are split per 8-h-tile block on the ACT ring so C's first
        matmuls issue right as B's last g tile lands.
  - host: concatenate the 8 token shards, reshape to [4, 4096, 2048].

All arithmetic (scales, ternarization, matmuls) happens on device; the host
only reshapes / transposes / pads / slices / concatenates.
"""

import sys
from contextlib import ExitStack

import numpy as np

if "/opt/trn_rl_repo" not in sys.path:
    sys.path.insert(0, "/opt/trn_rl_repo")

import concourse.bass as bass  # noqa: E402,F401
import concourse.mybir as mybir  # noqa: E402
import concourse.tile as tile  # noqa: E402
from concourse import bacc  # noqa: E402

F32 = mybir.dt.float32
BF16 = mybir.dt.bfloat16
FP8 = mybir.dt.float8e4
AF = mybir.ActivationFunctionType
ALU = mybir.AluOpType
AX = mybir.AxisListType

# Full problem geometry (hardcoded per contest rules).
B, S, D = 4, 4096, 2048
H_REAL = 5461
HP = 5504            # H padded to 43*128
N_CORES = 8
M = (B * S) // N_CORES   # tokens per core = 2048


def build_module(d=D, m=M, hp=HP, n_cores=N_CORES, h_real=H_REAL,
                 hb=4, w13_dt=FP8, w2q_dt=FP8,
                 ag_chunk_tiles=(2, 3, 6, 8, 12, 12),
                 w2qk_pre=20, gq_block=8):
    """Build + compile the per-core SPMD Bass module."""
    kd = d // 128        # k-tiles over D
    ht = hp // 128       # h-tiles
    mc = m // 512        # m-chunks of 512 in phase B
    assert d % 128 == 0 and hp % 128 == 0 and m % 512 == 0
    n_true = h_real * d
    sw = d * hp // (n_cores * 128)   # w2 slice free elems per partition
    r13 = d // n_cores               # weight-slice rows (w1t/w3t)
    assert r13 % 128 == 0
    a13 = r13 // 128
    assert sum(ag_chunk_tiles) == ht

    nwch = 8                         # weight DMA / asum chunks
    chunk_h = hp // nwch             # 688
    assert hp % nwch == 0 and sw % nwch == 0
    c2w = sw // nwch                 # w2 asum chunk width (1376)

    chunks = []
    t0 = 0
    for n in ag_chunk_tiles:
        chunks.append((t0, n))
        t0 += n
    max_cnt = max(ag_chunk_tiles)

    groups = [list(range(n_cores))]

    nc = bacc.Bacc(
        "TRN2",
        target_bir_lowering=False,
        debug=False,
        num_devices=n_cores,
    )
    xT = nc.dram_tensor("xT", [d, m], F32, kind="ExternalInput").ap()
    wsh1 = nc.dram_tensor("wsh1", [r13, hp], F32, kind="ExternalInput").ap()
    wsh3 = nc.dram_tensor("wsh3", [r13, hp], F32, kind="ExternalInput").ap()
    wsh2 = nc.dram_tensor("wsh2", [128, sw], F32, kind="ExternalInput").ap()
    y = nc.dram_tensor("y", [m, d], F32, kind="ExternalOutput").ap()

    xview = xT.rearrange("(k p) m -> p k m", p=128)
    v1 = wsh1.rearrange("(a p) h -> p a h", p=128)   # [128, a13, hp]
    v3 = wsh3.rearrange("(a p) h -> p a h", p=128)

    with tile.TileContext(nc) as tc:
        with ExitStack() as ctx:
            dram = ctx.enter_context(tc.tile_pool(name="dram", bufs=1, space="DRAM"))
            g_dram = dram.tile([hp, m], BF16, tag="g", name="g")
            cc_pre_i = dram.tile([1, 8], F32, tag="ccpi", name="ccpi")
            cc_pre_o = dram.tile([1, 8], F32, tag="ccpo", name="ccpo")
            cc_in = dram.tile([1, 8], F32, tag="cc_in", name="cc_in")
            cc_out = dram.tile([1, 8], F32, tag="cc_out", name="cc_out")
            g_rd = g_dram.rearrange("(k p) m -> p k m", p=128)

            agin13 = []
            agout13 = []
            for q, (ct0, cnt) in enumerate(chunks):
                agin13.append(dram.tile(
                    [r13, 2, cnt * 128], w13_dt,
                    tag=f"agi{q}", name=f"agi{q}"))
                agout13.append(dram.tile(
                    [d, 2, cnt * 128], w13_dt,
                    tag=f"ago{q}", name=f"ago{q}"))
            agin2 = dram.tile([128, sw], w2q_dt, tag="agi2", name="agi2")
            agout2 = dram.tile([hp, d], w2q_dt,
                               tag="ago2", name="ago2")
            agout2_rd = agout2.rearrange("(k p) dd -> p k dd", p=128)

            pc = ctx.enter_context(tc.tile_pool(name="pconst", bufs=1))
            sx = ctx.enter_context(ExitStack())
            xp = sx.enter_context(tc.tile_pool(name="xp", bufs=1))
            xstg_p = sx.enter_context(tc.tile_pool(name="xstg", bufs=2))
            bias = {}

            # t=0: absorb the CC rendezvous cost while DMAs stream.
            nc.gpsimd.collective_compute(
                "AllReduce", ALU.add, replica_groups=groups,
                ins=[cc_pre_i.opt()], outs=[cc_pre_o.opt()],
            )

            x_sb = xp.tile([128, kd, m], BF16, tag="x_sb", name="x_sb")

            # ---------------- phase A: scales + ternarize -------------------
            with ExitStack() as sa:
                res_p = sa.enter_context(tc.tile_pool(name="res", bufs=1))
                w2s_p = sa.enter_context(tc.tile_pool(name="w2s", bufs=1))
                sc1_pool = sa.enter_context(tc.tile_pool(name="scale1", bufs=1))
                scps = sa.enter_context(
                    tc.tile_pool(name="scps", bufs=1, space="PSUM"))
                qq_p = sa.enter_context(tc.tile_pool(name="qq", bufs=2))

                # single-read staging: w1/w3 resident fp32 for the ternarize
                res = res_p.tile([128, 2, a13, hp], F32, tag="res", name="res")
                asum = sc1_pool.tile([128, 3, nwch], F32, tag="asum",
                                     name="asum")
                # x streams on the ACT ring in parallel with the weights.
                # bf16 casts run on GPSIMD (Pool) -- the only engine with
                # free FIFO slots during the scale chain -- split into groups
                # interleaved with the collective triggers so the xstg ring
                # slots recycle without stalling the x DMAs.
                xstg_t = []
                for k in range(kd):
                    xstg = xstg_p.tile([128, m], F32, tag=f"xstg{k % 2}",
                                       name=f"xstg{k % 2}")
                    nc.scalar.dma_start(xstg, xview[:, k, :])
                    xstg_t.append(xstg)

                def emit_casts(k0, k1):
                    for k in range(k0, k1):
                        nc.gpsimd.tensor_scalar_add(x_sb[:, k, :],
                                                    xstg_t[k], 0.0)

                emit_casts(0, 8)
                for c in range(nwch):
                    hs_ = slice(c * chunk_h, (c + 1) * chunk_h)
                    cs = slice(c * c2w, (c + 1) * c2w)
                    nc.sync.dma_start(res[:, 0, :, hs_], v1[:, :, hs_])
                    nc.sync.dma_start(res[:, 1, :, hs_], v3[:, :, hs_])
                    stg2 = w2s_p.tile([128, c2w], F32, tag="w2stg",
                                      name="w2stg")
                    nc.sync.dma_start(stg2, wsh2[:, cs])
                    nc.vector.tensor_reduce(
                        asum[:, 0:2, c], res[:, :, :, hs_], axis=AX.XY,
                        op=ALU.add, apply_absolute_value=True,
                    )
                    nc.vector.tensor_reduce(
                        asum[:, 2:3, c], stg2, axis=AX.X, op=ALU.add,
                        apply_absolute_value=True,
                    )

                part8 = sc1_pool.tile([128, 8], F32, tag="part8", name="part8")
                nc.vector.memset(part8, 0.0)
                nc.vector.tensor_reduce(part8[:, 0:3], asum, axis=AX.X,
                                        op=ALU.add)
                ones = sc1_pool.tile([128, 1], F32, tag="ones", name="ones")
                nc.vector.memset(ones, 1.0)
                ps = scps.tile([8, 1], F32, tag="scps", name="scps")
                nc.tensor.matmul(ps, lhsT=part8, rhs=ones, start=True,
                                 stop=True)
                ssum8 = sc1_pool.tile([8, 1], F32, tag="ssum8", name="ssum8")
                nc.scalar.copy(ssum8, ps)
                nc.sync.dma_start(cc_in.rearrange("a b -> b a"), ssum8)
                nc.gpsimd.collective_compute(
                    "AllReduce", ALU.add, replica_groups=groups,
                    ins=[cc_in.opt()], outs=[cc_out.opt()],
                )
                emit_casts(8, 12)
                g8 = sc1_pool.tile([1, 8], F32, tag="g8", name="g8")
                nc.sync.dma_start(g8, cc_out)
                gb = pc.tile([128, 8], F32, tag="gb", name="gb")
                nc.gpsimd.partition_broadcast(gb, g8)
                emit_casts(12, kd)
                for t, name in enumerate(["w1", "w3", "w2"]):
                    for sgn in ("p", "n"):
                        bias[name + sgn] = pc.tile(
                            [128, 1], F32, tag=f"b_{name}{sgn}",
                            name=f"b_{name}{sgn}")
                        k = 0.5 / n_true if sgn == "p" else -0.5 / n_true
                        nc.vector.tensor_scalar(
                            bias[name + sgn], gb[:, t:t + 1], k, None,
                            ALU.mult,
                        )
                s23 = pc.tile([128, 1], F32, tag="s23", name="s23")
                nc.vector.tensor_mul(s23, bias["w3p"], bias["w2p"])

                def quantize(pool, out_ap, stg_ap, bn, bp, dt, eng, fdim):
                    pr = stg_ap.shape[0]
                    fw = stg_ap.shape[-1]
                    if eng == "act":
                        qa = pool.tile([128, fdim], dt, tag=f"qa{fdim}",
                                       name=f"qa{fdim}")
                        qb = pool.tile([128, fdim], dt, tag=f"qb{fdim}",
                                       name=f"qb{fdim}")
                        nc.scalar.activation(qa[:pr, :fw], stg_ap, AF.Sign,
                                             bias=bn[:pr])
                        nc.scalar.activation(qb[:pr, :fw], stg_ap, AF.Sign,
                                             bias=bp[:pr])
                        nc.vector.tensor_add(out_ap, qa[:pr, :fw],
                                             qb[:pr, :fw])
                    else:
                        qa = pool.tile([128, fdim], dt, tag=f"da{fdim}",
                                       name=f"da{fdim}")
                        qb = pool.tile([128, fdim], dt, tag=f"db{fdim}",
                                       name=f"db{fdim}")
                        nc.vector.tensor_scalar(qa[:pr, :fw], stg_ap, bp[:pr],
                                                2.0, ALU.is_ge, ALU.mult)
                        nc.vector.tensor_scalar(qb[:pr, :fw], stg_ap, bn[:pr],
                                                2.0, ALU.is_lt, ALU.mult)
                        nc.vector.tensor_sub(out_ap, qa[:pr, :fw],
                                             qb[:pr, :fw])

                # shard-ternarize w1 (ACT) / w3 (DVE) straight from the
                # resident fp32 copy; AllGather per chunk, smallest first.
                for q, (ct0, cnt) in enumerate(chunks):
                    chw = cnt * 128
                    hsl = slice(ct0 * 128, ct0 * 128 + chw)
                    ring = nc.sync if q == 0 else nc.scalar
                    for a in range(a13):
                        rs = slice(a * 128, (a + 1) * 128)
                        for t in range(2):
                            qt = qq_p.tile([128, max_cnt * 128], w13_dt,
                                           tag=f"qt{t}", name=f"qt{t}")
                            quantize(qq_p, qt[:, :chw], res[:, t, a, hsl],
                                     bias["w1n" if t == 0 else "w3n"],
                                     bias["w1p" if t == 0 else "w3p"],
                                     w13_dt, "act" if t == 0 else "dve",
                                     max_cnt * 128)
                            ring.dma_start(agin13[q][rs, t, :], qt[:, :chw])
                    nc.gpsimd.collective_compute(
                        "AllGather", ALU.bypass, replica_groups=groups,
                        ins=[agin13[q].opt()], outs=[agout13[q].opt()],
                    )

            # ---------------- phase B ---------------------------------------
            with ExitStack() as sb:
                wq_p = sb.enter_context(tc.tile_pool(name="wq", bufs=2))
                sl_p = sb.enter_context(tc.tile_pool(name="slp", bufs=3))
                g_p = sb.enter_context(tc.tile_pool(name="gp", bufs=2))
                zps = sb.enter_context(
                    tc.tile_pool(name="zps", bufs=8, space="PSUM"))
                w2t_p = sb.enter_context(tc.tile_pool(name="w2t", bufs=2))
                qq2_p = sb.enter_context(tc.tile_pool(name="qq2", bufs=2))

                def emit_w2_ternarize():
                    # w2 ternarize + AllGather in the shadow of phase B
                    # (gates only phase C).
                    for c in range(nwch):
                        cs = slice(c * c2w, (c + 1) * c2w)
                        stg = w2t_p.tile([128, c2w], F32, tag="q2stg",
                                         name="q2stg")
                        nc.scalar.dma_start(stg, wsh2[:, cs])
                        qt = qq2_p.tile([128, c2w], w2q_dt, tag="q2t",
                                        name="q2t")
                        quantize(qq2_p, qt, stg, bias["w2n"], bias["w2p"],
                                 w2q_dt, "act" if c % 2 == 0 else "dve", c2w)
                        nc.scalar.dma_start(agin2[:, cs], qt)
                    nc.gpsimd.collective_compute(
                        "AllGather", ALU.bypass, replica_groups=groups,
                        ins=[agin2.opt()], outs=[agout2.opt()],
                    )

                for q, (ct0, cnt) in enumerate(chunks):
                    agov = agout13[q].rearrange(
                        "(k p) two h -> p k two h", p=128)
                    for b0 in range(0, cnt, hb):
                        nh = min(hb, cnt - b0)
                        hw = nh * 128
                        habs = (ct0 + b0) * 128       # absolute h start
                        wqb = []
                        for t in range(2):
                            wq_t = wq_p.tile([128, kd, hb * 128], w13_dt,
                                             tag=f"wq{t}", name=f"wq{t}")
                            nc.sync.dma_start(
                                wq_t[:, :, :hw],
                                agov[:, :, t, b0 * 128:b0 * 128 + hw])
                            wqb.append(wq_t)

                        for hti in range(nh):
                            hs = slice(hti * 128, (hti + 1) * 128)
                            g_t = g_p.tile([128, m], BF16, tag="g_t",
                                           name="g_t")
                            zz = [[None] * mc, [None] * mc]
                            for t in range(2):
                                for mci in range(mc):
                                    zz[t][mci] = zps.tile([128, 512], F32,
                                                          tag="z", name="z")
                                for k in range(kd):
                                    for mci in range(mc):
                                        nc.tensor.matmul(
                                            zz[t][mci], lhsT=wqb[t][:, k, hs],
                                            rhs=x_sb[:, k,
                                                     mci * 512:(mci + 1) * 512],
                                            start=(k == 0), stop=(k == kd - 1),
                                        )
                            for mci in range(mc):
                                ms = slice(mci * 512, (mci + 1) * 512)
                                sl = sl_p.tile([128, 512], BF16, tag="sl",
                                               name="sl")
                                nc.scalar.activation(sl, zz[0][mci], AF.Silu,
                                                     bias=0.0,
                                                     scale=bias["w1p"])
                                sc = sl_p.tile([128, 512], BF16, tag="sc",
                                               name="sc")
                                nc.scalar.activation(sc, zz[1][mci], AF.Copy,
                                                     bias=0.0, scale=s23)
                                nc.vector.tensor_mul(g_t[:, ms], sl, sc)
                            nc.sync.dma_start(
                                g_dram[habs + hti * 128:
                                       habs + (hti + 1) * 128, :], g_t)

                    if q == 1:
                        emit_w2_ternarize()

            sx.close()   # free x pools before phase C allocates

            # ---------------- phase C ---------------------------------------
            ndc = (d + 511) // 512
            with ExitStack() as scx:
                w2b_p = scx.enter_context(tc.tile_pool(name="w2b", bufs=1))
                gq_p = scx.enter_context(tc.tile_pool(name="gq", bufs=3))
                y_p = scx.enter_context(tc.tile_pool(name="yp", bufs=2 * ndc))
                yps = scx.enter_context(
                    tc.tile_pool(name="yps", bufs=8, space="PSUM"))

                w2qk = [None] * ht
                for k2 in range(ht):
                    t_ = w2b_p.tile([128, d], w2q_dt, tag=f"w2b{k2}",
                                    name=f"w2b{k2}")
                    nc.scalar.dma_start(t_, agout2_rd[:, k2, :])
                    w2qk[k2] = t_
                for mt in range(m // 128):
                    ms = slice(mt * 128, (mt + 1) * 128)
                    gq = gq_p.tile([128, ht, 128], BF16, tag="gq", name="gq")
                    for b0 in range(0, ht, gq_block):
                        b1 = min(b0 + gq_block, ht)
                        nc.scalar.dma_start(gq[:, b0:b1, :],
                                            g_rd[:, b0:b1, ms])
                    yp4 = [yps.tile([128, 512], F32, tag="yps", name="yps")
                           for _ in range(ndc)]
                    for k2 in range(ht):
                        for di, dc0 in enumerate(range(0, d, 512)):
                            nd = min(512, d - dc0)
                            nc.tensor.matmul(
                                yp4[di][:, :nd],
                                lhsT=gq[:, k2, :],
                                rhs=w2qk[k2][:, dc0:dc0 + nd],
                                start=(k2 == 0), stop=(k2 == ht - 1),
                            )
                    for di, dc0 in enumerate(range(0, d, 512)):
                        nd = min(512, d - dc0)
                        ysb = y_p.tile([128, 512], F32, tag="ysb", name="ysb")
                        nc.scalar.copy(ysb[:, :nd], yp4[di][:, :nd])
                        nc.sync.dma_start(
                            y[mt * 128:(mt + 1) * 128, dc0:dc0 + nd],
                            ysb[:, :nd])

    nc.compile()
    return nc


_NC_CACHE = {}


def _get_module():
    if "nc" not in _NC_CACHE:
        _NC_CACHE["nc"] = build_module()
    return _NC_CACHE["nc"]


def prep_inputs(x, w1, w3, w2, d=D, m=M, hp=HP, n_cores=N_CORES):
    """Host-side layout work: pad, transpose, shard, slice. No arithmetic."""
    h_real = w1.shape[0]
    x = np.ascontiguousarray(np.asarray(x, dtype=np.float32))
    xf = x.reshape(-1, d)
    w1t = np.zeros((d, hp), np.float32)
    w1t[:, :h_real] = np.asarray(w1, np.float32).T
    w3t = np.zeros((d, hp), np.float32)
    w3t[:, :h_real] = np.asarray(w3, np.float32).T
    w2t = np.zeros((hp, d), np.float32)
    w2t[:h_real, :] = np.asarray(w2, np.float32).T

    r13 = d // n_cores
    r2 = hp // n_cores
    sw = d * hp // (n_cores * 128)

    in_maps = []
    for c in range(n_cores):
        xc = np.ascontiguousarray(xf[c * m:(c + 1) * m].T)   # [d, m]
        in_maps.append({
            "xT": xc,
            "wsh1": np.ascontiguousarray(w1t[c * r13:(c + 1) * r13]),
            "wsh3": np.ascontiguousarray(w3t[c * r13:(c + 1) * r13]),
            "wsh2": np.ascontiguousarray(
                w2t[c * r2:(c + 1) * r2].reshape(128, sw)),
        })
    return in_maps


def kernel(x, w1, w3, w2):
    from concourse.bass_utils import run_bass_kernel_spmd

    nc = _get_module()
    in_maps = prep_inputs(x, w1, w3, w2)
    res = run_bass_kernel_spmd(nc, in_maps, core_ids=list(range(N_CORES)))
    _NC_CACHE["last_results"] = res
    yf = np.concatenate([r["y"] for r in res.results], axis=0)  # [16384, 2048]
    return np.ascontiguousarray(yf.reshape(B, S, D).astype(np.float32))


# revision 17
# speedup vs baseline: 1.2329x; 1.2329x over previous
"""Trainium2 Bass kernel for nn_BinaryMLP (BitNet-ternary SwiGLU MLP).

reference math (fp32):
    s_i = mean(|w_i|)            (per-tensor scalar, i in {1,3,2})
    wq_i = clip(round(w_i/s_i), -1, 1) * s_i     (ternary * scale)
    h1 = x @ w1q.T ; h3 = x @ w3q.T
    y  = (silu(h1) * h3) @ w2q.T

Strategy (8 cores, data-parallel over the 16384 tokens):
  - host: pad H 5461->5504, transpose x / w1 / w3 / w2 into contraction-major
    layouts (pure layout work, no arithmetic), split tokens 8 ways, and give
    each core a distinct 1/8 row-slice of each weight tensor.
  - device (per core, identical SPMD program):
      phase A (head, target <90us):
        * dummy AllReduce at t=0 absorbs the one-time CC rendezvous cost.
        * w1/w3 fp32 slices stream once into a resident SBUF tile on the SP
          DMA ring while DVE abs-reduces partial |w| sums per chunk; w2
          streams through a small staging pool (re-read later for its own
          ternarize, off the critical path).  x streams in parallel on the
          ACT DMA ring and is cast to bf16 on DVE.
        * tiny 8-float AllReduce -> ternarization thresholds +-s/2.
        * ternarize w1 (ACT Sign) / w3 (DVE is_ge/is_lt) from the resident
          fp32 copy -- no second DRAM read.  AllGather chunks are ordered
          smallest-first (2,3,6,8,12,12 h-tiles) so phase B starts on the
          first chunk ASAP; later chunks pipeline behind B's compute.
      phase B: h1/h3 matmuls vs resident bf16 x (fp8 ternary weights
        stationary, 8 psum banks), g = silu((s1/2) z1) * ((s3 s2/4) z3)
        -> bf16 -> DRAM.  w2's ternarize + AllGather + a partial prefetch
        of its fp8 tiles run in the shadow of B.
      phase C: y[m,d] = sum_h g[h,m] t2[h,d], g stationary, fp32 PSUM.
        gq loads are split per 8-h-tile block on the ACT ring so C's first
        matmuls issue right as B's last g tile lands.
  - host: concatenate the 8 token shards, reshape to [4, 4096, 2048].

All arithmetic (scales, ternarization, matmuls) happens on device; the host
only reshapes / transposes / pads / slices / concatenates.
"""

import sys
from contextlib import ExitStack

import numpy as np

if "/opt/trn_rl_repo" not in sys.path:
    sys.path.insert(0, "/opt/trn_rl_repo")

import concourse.bass as bass  # noqa: E402,F401
import concourse.mybir as mybir  # noqa: E402
import concourse.tile as tile  # noqa: E402
from concourse import bacc  # noqa: E402

F32 = mybir.dt.float32
BF16 = mybir.dt.bfloat16
FP8 = mybir.dt.float8e4
AF = mybir.ActivationFunctionType
ALU = mybir.AluOpType
AX = mybir.AxisListType

# Full problem geometry (hardcoded per contest rules).
B, S, D = 4, 4096, 2048
H_REAL = 5461
HP = 5504            # H padded to 43*128
N_CORES = 8
M = (B * S) // N_CORES   # tokens per core = 2048


def build_module(d=D, m=M, hp=HP, n_cores=N_CORES, h_real=H_REAL,
                 hb=4, w13_dt=FP8, w2q_dt=FP8,
                 ag_chunk_tiles=(2, 3, 6, 8, 12, 12),
                 w2qk_pre=20, gq_block=8):
    """Build + compile the per-core SPMD Bass module."""
    kd = d // 128        # k-tiles over D
    ht = hp // 128       # h-tiles
    mc = m // 512        # m-chunks of 512 in phase B
    assert d % 128 == 0 and hp % 128 == 0 and m % 512 == 0
    n_true = h_real * d
    sw = d * hp // (n_cores * 128)   # w2 slice free elems per partition
    r13 = d // n_cores               # weight-slice rows (w1t/w3t)
    assert r13 % 128 == 0
    a13 = r13 // 128
    assert sum(ag_chunk_tiles) == ht

    nwch = 8                         # weight DMA / asum chunks
    chunk_h = hp // nwch             # 688
    assert hp % nwch == 0 and sw % nwch == 0
    c2w = sw // nwch                 # w2 asum chunk width (1376)

    chunks = []
    t0 = 0
    for n in ag_chunk_tiles:
        chunks.append((t0, n))
        t0 += n
    max_cnt = max(ag_chunk_tiles)

    groups = [list(range(n_cores))]

    nc = bacc.Bacc(
        "TRN2",
        target_bir_lowering=False,
        debug=False,
        num_devices=n_cores,
    )
    xT = nc.dram_tensor("xT", [d, m], F32, kind="ExternalInput").ap()
    wsh1 = nc.dram_tensor("wsh1", [r13, hp], F32, kind="ExternalInput").ap()
    wsh3 = nc.dram_tensor("wsh3", [r13, hp], F32, kind="ExternalInput").ap()
    wsh2 = nc.dram_tensor("wsh2", [128, sw], F32, kind="ExternalInput").ap()
    y = nc.dram_tensor("y", [m, d], F32, kind="ExternalOutput").ap()

    xview = xT.rearrange("(k p) m -> p k m", p=128)
    v1 = wsh1.rearrange("(a p) h -> p a h", p=128)   # [128, a13, hp]
    v3 = wsh3.rearrange("(a p) h -> p a h", p=128)

    with tile.TileContext(nc) as tc:
        with ExitStack() as ctx:
            dram = ctx.enter_context(tc.tile_pool(name="dram", bufs=1, space="DRAM"))
            g_dram = dram.tile([hp, m], BF16, tag="g", name="g")
            cc_pre_i = dram.tile([1, 8], F32, tag="ccpi", name="ccpi")
            cc_pre_o = dram.tile([1, 8], F32, tag="ccpo", name="ccpo")
            cc_in = dram.tile([1, 8], F32, tag="cc_in", name="cc_in")
            cc_out = dram.tile([1, 8], F32, tag="cc_out", name="cc_out")
            g_rd = g_dram.rearrange("(k p) m -> p k m", p=128)

            agin13 = []
            agout13 = []
            for q, (ct0, cnt) in enumerate(chunks):
                agin13.append(dram.tile(
                    [r13, 2, cnt * 128], w13_dt,
                    tag=f"agi{q}", name=f"agi{q}"))
                agout13.append(dram.tile(
                    [d, 2, cnt * 128], w13_dt,
                    tag=f"ago{q}", name=f"ago{q}"))
            agin2 = dram.tile([128, sw], w2q_dt, tag="agi2", name="agi2")
            agout2 = dram.tile([hp, d], w2q_dt,
                               tag="ago2", name="ago2")
            agout2_rd = agout2.rearrange("(k p) dd -> p k dd", p=128)

            pc = ctx.enter_context(tc.tile_pool(name="pconst", bufs=1))
            sx = ctx.enter_context(ExitStack())
            xp = sx.enter_context(tc.tile_pool(name="xp", bufs=1))
            xstg_p = sx.enter_context(tc.tile_pool(name="xstg", bufs=2))
            bias = {}

            # t=0: absorb the CC rendezvous cost while DMAs stream.
            nc.gpsimd.collective_compute(
                "AllReduce", ALU.add, replica_groups=groups,
                ins=[cc_pre_i.opt()], outs=[cc_pre_o.opt()],
            )

            x_sb = xp.tile([128, kd, m], BF16, tag="x_sb", name="x_sb")

            # ---------------- phase A: scales + ternarize -------------------
            with ExitStack() as sa:
                res_p = sa.enter_context(tc.tile_pool(name="res", bufs=1))
                w2s_p = sa.enter_context(tc.tile_pool(name="w2s", bufs=1))
                sc1_pool = sa.enter_context(tc.tile_pool(name="scale1", bufs=1))
                scps = sa.enter_context(
                    tc.tile_pool(name="scps", bufs=1, space="PSUM"))
                qq_p = sa.enter_context(tc.tile_pool(name="qq", bufs=2))

                # single-read staging: w1/w3 resident fp32 for the ternarize
                res = res_p.tile([128, 2, a13, hp], F32, tag="res", name="res")
                asum = sc1_pool.tile([128, 3, nwch], F32, tag="asum",
                                     name="asum")
                # x streams on the SP ring behind the weight reads (weights
                # gate the scale AllReduce -> first).  bf16 casts run on DVE,
                # emitted after the chunk-0 ternarize so they don't delay the
                # first AllGather on the DVE FIFO.
                xstg_t = []

                def emit_casts(k0, k1):
                    for k in range(k0, k1):
                        nc.vector.tensor_scalar_add(x_sb[:, k, :],
                                                    xstg_t[k], 0.0)
                for c in range(nwch):
                    hs_ = slice(c * chunk_h, (c + 1) * chunk_h)
                    cs = slice(c * c2w, (c + 1) * c2w)
                    nc.sync.dma_start(res[:, 0, :, hs_], v1[:, :, hs_])
                    nc.sync.dma_start(res[:, 1, :, hs_], v3[:, :, hs_])
                    stg2 = w2s_p.tile([128, c2w], F32, tag="w2stg",
                                      name="w2stg")
                    nc.sync.dma_start(stg2, wsh2[:, cs])
                    nc.vector.tensor_reduce(
                        asum[:, 0:2, c], res[:, :, :, hs_], axis=AX.XY,
                        op=ALU.add, apply_absolute_value=True,
                    )
                    nc.vector.tensor_reduce(
                        asum[:, 2:3, c], stg2, axis=AX.X, op=ALU.add,
                        apply_absolute_value=True,
                    )

                for k in range(kd):
                    xstg = xstg_p.tile([128, m], F32, tag=f"xstg{k % 2}",
                                       name=f"xstg{k % 2}")
                    nc.sync.dma_start(xstg, xview[:, k, :])
                    xstg_t.append(xstg)

                part8 = sc1_pool.tile([128, 8], F32, tag="part8", name="part8")
                nc.vector.memset(part8, 0.0)
                nc.vector.tensor_reduce(part8[:, 0:3], asum, axis=AX.X,
                                        op=ALU.add)
                ones = sc1_pool.tile([128, 1], F32, tag="ones", name="ones")
                nc.vector.memset(ones, 1.0)
                ps = scps.tile([8, 1], F32, tag="scps", name="scps")
                nc.tensor.matmul(ps, lhsT=part8, rhs=ones, start=True,
                                 stop=True)
                ssum8 = sc1_pool.tile([8, 1], F32, tag="ssum8", name="ssum8")
                nc.scalar.copy(ssum8, ps)
                nc.scalar.dma_start(cc_in.rearrange("a b -> b a"), ssum8)
                nc.gpsimd.collective_compute(
                    "AllReduce", ALU.add, replica_groups=groups,
                    ins=[cc_in.opt()], outs=[cc_out.opt()],
                )
                g8 = sc1_pool.tile([1, 8], F32, tag="g8", name="g8")
                nc.scalar.dma_start(g8, cc_out)
                gb = pc.tile([128, 8], F32, tag="gb", name="gb")
                nc.gpsimd.partition_broadcast(gb, g8)
                for t, name in enumerate(["w1", "w3", "w2"]):
                    for sgn in ("p", "n"):
                        bias[name + sgn] = pc.tile(
                            [128, 1], F32, tag=f"b_{name}{sgn}",
                            name=f"b_{name}{sgn}")
                        k = 0.5 / n_true if sgn == "p" else -0.5 / n_true
                        nc.vector.tensor_scalar(
                            bias[name + sgn], gb[:, t:t + 1], k, None,
                            ALU.mult,
                        )
                s23 = pc.tile([128, 1], F32, tag="s23", name="s23")
                nc.vector.tensor_mul(s23, bias["w3p"], bias["w2p"])

                def quantize(pool, out_ap, stg_ap, bn, bp, dt, eng, fdim):
                    pr = stg_ap.shape[0]
                    fw = stg_ap.shape[-1]
                    if eng == "act":
                        qa = pool.tile([128, fdim], dt, tag=f"qa{fdim}",
                                       name=f"qa{fdim}")
                        qb = pool.tile([128, fdim], dt, tag=f"qb{fdim}",
                                       name=f"qb{fdim}")
                        nc.scalar.activation(qa[:pr, :fw], stg_ap, AF.Sign,
                                             bias=bn[:pr])
                        nc.scalar.activation(qb[:pr, :fw], stg_ap, AF.Sign,
                                             bias=bp[:pr])
                        nc.vector.tensor_add(out_ap, qa[:pr, :fw],
                                             qb[:pr, :fw])
                    else:
                        qa = pool.tile([128, fdim], dt, tag=f"da{fdim}",
                                       name=f"da{fdim}")
                        qb = pool.tile([128, fdim], dt, tag=f"db{fdim}",
                                       name=f"db{fdim}")
                        nc.vector.tensor_scalar(qa[:pr, :fw], stg_ap, bp[:pr],
                                                2.0, ALU.is_ge, ALU.mult)
                        nc.vector.tensor_scalar(qb[:pr, :fw], stg_ap, bn[:pr],
                                                2.0, ALU.is_lt, ALU.mult)
                        nc.vector.tensor_sub(out_ap, qa[:pr, :fw],
                                             qb[:pr, :fw])

                # shard-ternarize w1 (ACT) / w3 (DVE) straight from the
                # resident fp32 copy; AllGather per chunk, smallest first.
                # chunk 0 goes first (its agin writes on the free ACT ring),
                # then the x casts slot into DVE, then the remaining chunks
                # (agin writes on the SP ring behind the x loads).
                def emit_tern_chunk(q):
                    ct0, cnt = chunks[q]
                    chw = cnt * 128
                    hsl = slice(ct0 * 128, ct0 * 128 + chw)
                    ring = nc.scalar if q == 0 else nc.sync
                    for a in range(a13):
                        rs = slice(a * 128, (a + 1) * 128)
                        for t in range(2):
                            qt = qq_p.tile([128, max_cnt * 128], w13_dt,
                                           tag=f"qt{t}", name=f"qt{t}")
                            quantize(qq_p, qt[:, :chw], res[:, t, a, hsl],
                                     bias["w1n" if t == 0 else "w3n"],
                                     bias["w1p" if t == 0 else "w3p"],
                                     w13_dt, "act" if t == 0 else "dve",
                                     max_cnt * 128)
                            ring.dma_start(agin13[q][rs, t, :], qt[:, :chw])
                    nc.gpsimd.collective_compute(
                        "AllGather", ALU.bypass, replica_groups=groups,
                        ins=[agin13[q].opt()], outs=[agout13[q].opt()],
                    )

                emit_tern_chunk(0)
                emit_casts(0, kd)
                for q in range(1, len(chunks)):
                    emit_tern_chunk(q)

            # ---------------- phase B ---------------------------------------
            with ExitStack() as sb:
                wq_p = sb.enter_context(tc.tile_pool(name="wq", bufs=2))
                sl_p = sb.enter_context(tc.tile_pool(name="slp", bufs=3))
                g_p = sb.enter_context(tc.tile_pool(name="gp", bufs=2))
                zps = sb.enter_context(
                    tc.tile_pool(name="zps", bufs=8, space="PSUM"))
                w2t_p = sb.enter_context(tc.tile_pool(name="w2t", bufs=2))
                qq2_p = sb.enter_context(tc.tile_pool(name="qq2", bufs=2))

                def emit_w2_ternarize():
                    # w2 ternarize + AllGather in the shadow of phase B
                    # (gates only phase C).
                    for c in range(nwch):
                        cs = slice(c * c2w, (c + 1) * c2w)
                        stg = w2t_p.tile([128, c2w], F32, tag="q2stg",
                                         name="q2stg")
                        nc.scalar.dma_start(stg, wsh2[:, cs])
                        qt = qq2_p.tile([128, c2w], w2q_dt, tag="q2t",
                                        name="q2t")
                        quantize(qq2_p, qt, stg, bias["w2n"], bias["w2p"],
                                 w2q_dt, "act" if c % 2 == 0 else "dve", c2w)
                        nc.scalar.dma_start(agin2[:, cs], qt)
                    nc.gpsimd.collective_compute(
                        "AllGather", ALU.bypass, replica_groups=groups,
                        ins=[agin2.opt()], outs=[agout2.opt()],
                    )

                for q, (ct0, cnt) in enumerate(chunks):
                    agov = agout13[q].rearrange(
                        "(k p) two h -> p k two h", p=128)
                    for b0 in range(0, cnt, hb):
                        nh = min(hb, cnt - b0)
                        hw = nh * 128
                        habs = (ct0 + b0) * 128       # absolute h start
                        wqb = []
                        for t in range(2):
                            wq_t = wq_p.tile([128, kd, hb * 128], w13_dt,
                                             tag=f"wq{t}", name=f"wq{t}")
                            nc.scalar.dma_start(
                                wq_t[:, :, :hw],
                                agov[:, :, t, b0 * 128:b0 * 128 + hw])
                            wqb.append(wq_t)

                        for hti in range(nh):
                            hs = slice(hti * 128, (hti + 1) * 128)
                            g_t = g_p.tile([128, m], BF16, tag="g_t",
                                           name="g_t")
                            zz = [[None] * mc, [None] * mc]
                            for t in range(2):
                                for mci in range(mc):
                                    zz[t][mci] = zps.tile([128, 512], F32,
                                                          tag="z", name="z")
                                for k in range(kd):
                                    for mci in range(mc):
                                        nc.tensor.matmul(
                                            zz[t][mci], lhsT=wqb[t][:, k, hs],
                                            rhs=x_sb[:, k,
                                                     mci * 512:(mci + 1) * 512],
                                            start=(k == 0), stop=(k == kd - 1),
                                        )
                            for mci in range(mc):
                                ms = slice(mci * 512, (mci + 1) * 512)
                                sl = sl_p.tile([128, 512], BF16, tag="sl",
                                               name="sl")
                                nc.scalar.activation(sl, zz[0][mci], AF.Silu,
                                                     bias=0.0,
                                                     scale=bias["w1p"])
                                sc = sl_p.tile([128, 512], BF16, tag="sc",
                                               name="sc")
                                nc.scalar.activation(sc, zz[1][mci], AF.Copy,
                                                     bias=0.0, scale=s23)
                                nc.vector.tensor_mul(g_t[:, ms], sl, sc)
                            nc.sync.dma_start(
                                g_dram[habs + hti * 128:
                                       habs + (hti + 1) * 128, :], g_t)

                    if q == 1:
                        emit_w2_ternarize()

            sx.close()   # free x pools before phase C allocates

            # ---------------- phase C ---------------------------------------
            ndc = (d + 511) // 512
            with ExitStack() as scx:
                w2b_p = scx.enter_context(tc.tile_pool(name="w2b", bufs=1))
                gq_p = scx.enter_context(tc.tile_pool(name="gq", bufs=3))
                y_p = scx.enter_context(tc.tile_pool(name="yp", bufs=2 * ndc))
                yps = scx.enter_context(
                    tc.tile_pool(name="yps", bufs=8, space="PSUM"))

                w2qk = [None] * ht
                for k2 in range(ht):
                    t_ = w2b_p.tile([128, d], w2q_dt, tag=f"w2b{k2}",
                                    name=f"w2b{k2}")
                    nc.scalar.dma_start(t_, agout2_rd[:, k2, :])
                    w2qk[k2] = t_
                for mt in range(m // 128):
                    ms = slice(mt * 128, (mt + 1) * 128)
                    gq = gq_p.tile([128, ht, 128], BF16, tag="gq", name="gq")
                    for b0 in range(0, ht, gq_block):
                        b1 = min(b0 + gq_block, ht)
                        nc.scalar.dma_start(gq[:, b0:b1, :],
                                            g_rd[:, b0:b1, ms])
                    yp4 = [yps.tile([128, 512], F32, tag="yps", name="yps")
                           for _ in range(ndc)]
                    for k2 in range(ht):
                        for di, dc0 in enumerate(range(0, d, 512)):
                            nd = min(512, d - dc0)
                            nc.tensor.matmul(
                                yp4[di][:, :nd],
                                lhsT=gq[:, k2, :],
                                rhs=w2qk[k2][:, dc0:dc0 + nd],
                                start=(k2 == 0), stop=(k2 == ht - 1),
                            )
                    for di, dc0 in enumerate(range(0, d, 512)):
                        nd = min(512, d - dc0)
                        ysb = y_p.tile([128, 512], F32, tag="ysb", name="ysb")
                        nc.scalar.copy(ysb[:, :nd], yp4[di][:, :nd])
                        nc.sync.dma_start(
                            y[mt * 128:(mt + 1) * 128, dc0:dc0 + nd],
                            ysb[:, :nd])

    nc.compile()
    return nc


_NC_CACHE = {}


def _get_module():
    if "nc" not in _NC_CACHE:
        _NC_CACHE["nc"] = build_module()
    return _NC_CACHE["nc"]


def prep_inputs(x, w1, w3, w2, d=D, m=M, hp=HP, n_cores=N_CORES):
    """Host-side layout work: pad, transpose, shard, slice. No arithmetic."""
    h_real = w1.shape[0]
    x = np.ascontiguousarray(np.asarray(x, dtype=np.float32))
    xf = x.reshape(-1, d)
    w1t = np.zeros((d, hp), np.float32)
    w1t[:, :h_real] = np.asarray(w1, np.float32).T
    w3t = np.zeros((d, hp), np.float32)
    w3t[:, :h_real] = np.asarray(w3, np.float32).T
    w2t = np.zeros((hp, d), np.float32)
    w2t[:h_real, :] = np.asarray(w2, np.float32).T

    r13 = d // n_cores
    r2 = hp // n_cores
    sw = d * hp // (n_cores * 128)

    in_maps = []
    for c in range(n_cores):
        xc = np.ascontiguousarray(xf[c * m:(c + 1) * m].T)   # [d, m]
        in_maps.append({
            "xT": xc,
            "wsh1": np.ascontiguousarray(w1t[c * r13:(c + 1) * r13]),
            "wsh3": np.ascontiguousarray(w3t[c * r13:(c + 1) * r13]),
            "wsh2": np.ascontiguousarray(
                w2t[c * r2:(c + 1) * r2].reshape(128, sw)),
        })
    return in_maps


def kernel(x, w1, w3, w2):
    from concourse.bass_utils import run_bass_kernel_spmd

    nc = _get_module()
    in_maps = prep_inputs(x, w1, w3, w2)
    res = run_bass_kernel_spmd(nc, in_maps, core_ids=list(range(N_CORES)))
    _NC_CACHE["last_results"] = res
    yf = np.concatenate([r["y"] for r in res.results], axis=0)  # [16384, 2048]
    return np.ascontiguousarray(yf.reshape(B, S, D).astype(np.float32))
